# revision 24
# baseline (speedup 1.0000x reference)
"""Cosine loss kernel for Trainium2 (8 NeuronCores, SPMD data-parallel).

loss = mean(1 - logits[i, labels[i]] / max(||logits[i]||, eps))

Sharding: rows split evenly across 8 cores. Each core streams its
[16384, 1000] f32 shard through SBUF in [128, 1000] tiles and computes,
per row i:
  dot_i  = logits[i, labels[i]]   (one fused DVE scalar_tensor_tensor:
                                   (iota == label_i) * x, accumulated)
  ssq_i  = sum(logits[i, :]^2)    (one ACT Square activation w/ accum)
then a small tail computes sum(cos_i) per partition. Host sums the 8
[128, 1] partials and finishes 1 - total/N.
"""

import os
import sys

import numpy as np

try:
    import concourse.bass as bass  # noqa: F401
except ImportError:
    for _p in ("/opt/trn_rl_repo", "/root/.axon_site/_ro/trn_rl_repo"):
        if os.path.isdir(_p) and _p not in sys.path:
            sys.path.insert(0, _p)
    import concourse.bass as bass

import concourse.mybir as mybir
from concourse import tile
from concourse.bass_utils import run_bass_kernel_spmd

N, C = 131072, 1000
N_CORES = 8
P = 128
ROWS_PER_CORE = N // N_CORES          # 16384
NTILES = ROWS_PER_CORE // P           # 128
EPS = 1e-8
F32 = mybir.dt.float32

_X_BUFS = 8


def build_nc(ntiles: int = NTILES) -> bass.Bass:
    nc = bass.Bass()
    rows = ntiles * P
    logits_in = nc.declare_dram_parameter("logits", [rows, C], F32, isOutput=False)
    # aux = [iota (C cols) | labels (ntiles cols)] fused into one tensor so
    # the first STT only ever waits on 2 DMA sems (aux + x) — the HW
    # instruction encoding has a small sync-wait budget.
    aux_in = nc.declare_dram_parameter("aux", [P, C + ntiles], F32, isOutput=False)
    out_ext = nc.declare_dram_parameter("partial", [P, 1], F32, isOutput=True)

    with tile.TileContext(nc) as tc:
        with (
            tc.tile_pool(name="const", bufs=1) as cpool,
            tc.tile_pool(name="x", bufs=_X_BUFS) as xpool,
            tc.tile_pool(name="scratch", bufs=2) as spool,
            tc.tile_pool(name="acc", bufs=1) as apool,
        ):
            aux_t = cpool.tile([P, C + ntiles], F32, tag="aux")
            nc.sync.dma_start(aux_t[:], aux_in[:])
            iota_t = aux_t[:, :C]
            # Pre-touch aux on DVE: the STT encoding has a tiny sync-wait
            # budget, so let this copy absorb the aux-DMA wait — every STT
            # then only waits on its x-tile DMA.
            labels_t = cpool.tile([P, ntiles], F32, tag="labels")
            nc.vector.tensor_copy(labels_t[:], aux_t[:, C:])

            dot_acc = apool.tile([P, ntiles], F32, tag="dot")
            ssq_acc = apool.tile([P, ntiles], F32, tag="ssq")

            # Each HW instruction can encode at most ONE distinct wait-sem
            # (same-sem requirements merge; distinct sems don't). Loop
            # structure enforcing that invariant:
            #  - x-DMAs issue from the scalar sequencer (HWDGE waits run at
            #    the issuing sequencer, so all ACT-side release/WAR
            #    requirements are same-proc and free);
            #  - the DMA refilling a slot is issued right after the Square
            #    that last read it (same 8-lane rotation position, so the
            #    ring-reuse lane wait is already observed by that Square's
            #    own wait) and carries only the DVE WAR sem;
            #  - per-tile one-column "drain absorber" copies carry the
            #    engine-sem waits (reading the persistent accumulators, not
            #    pooled scratch, to stay out of slot-release bookkeeping),
            #    so STT/Square carry only their x-DMA lane wait.
            xs = []
            dma_insts = []
            for t in range(min(_X_BUFS, ntiles)):
                x = xpool.tile([P, C], F32, tag="x")
                d = nc.scalar.dma_start(x[:], logits_in[t * P:(t + 1) * P, :])
                xs.append(x)
                dma_insts.append(d)
            # Lane priming: one-off scalar copies observing every DMA lane
            # sem up front, so the first ring-reuse refill has its lane
            # requirement pre-covered whatever the lane numbering is.
            prime = cpool.tile([P, len(xs) + 1], F32, tag="prime")
            for k, xk in enumerate(xs):
                nc.scalar.copy(prime[:, k:k + 1], xk[:, :1])
            # aux's DMA lane is part of the same global 8-lane rotation; the
            # first ring-reuse refill inherits it, so observe it here too.
            nc.scalar.copy(prime[:, len(xs):len(xs) + 1], aux_t[:, :1])

            for t in range(ntiles):
                x = xs[t]
                pa = spool.tile([P, 1], F32, tag="pa")
                nc.vector.tensor_copy(
                    pa[:], labels_t[:, :1] if t == 0 else dot_acc[:, t - 1:t]
                )
                stt_scr = spool.tile([P, C], F32, tag="stt")
                nc.vector.scalar_tensor_tensor(
                    out=stt_scr[:],
                    in0=iota_t,
                    scalar=labels_t[:, t:t + 1],
                    in1=x[:],
                    op0=mybir.AluOpType.is_equal,
                    op1=mybir.AluOpType.mult,
                    accum_out=dot_acc[:, t:t + 1],
                )

                if t > 0:
                    pc = spool.tile([P, 1], F32, tag="pc")
                    pc_i = nc.scalar.copy(pc[:], ssq_acc[:, t - 1:t])
                    # Refill the slot freed at t-1: pc_t's Act-sem wait
                    # (>= Square_{t-1}) covers the WAR, and Square_{t-1}
                    # already observed that slot's DMA lane sem, so the
                    # refill carries only the DVE WAR sem.
                    tn = t - 1 + _X_BUFS
                    if tn < ntiles:
                        xn = xpool.tile([P, C], F32, tag="x")
                        dma_i = nc.scalar.dma_start(
                            xn[:], logits_in[tn * P:(tn + 1) * P, :]
                        )
                        tile.add_dep_helper(
                            dma_i.ins,
                            pc_i.ins,
                            sync=False,
                            reason="x-slot refill ordered after pc absorber",
                        )
                        xs.append(xn)
                        dma_insts.append(dma_i)
                act_scr = spool.tile([P, C], F32, tag="act")
                sq = nc.scalar.activation(
                    out=act_scr[:],
                    in_=x[:],
                    func=mybir.ActivationFunctionType.Square,
                    accum_out=ssq_acc[:, t:t + 1],
                )

            # Tail: cos = dot / max(sqrt(ssq), EPS); out = per-partition sum(cos)
            norm = apool.tile([P, ntiles], F32, tag="norm")
            sqrt_i = nc.scalar.activation(
                out=norm[:], in_=ssq_acc[:], func=mybir.ActivationFunctionType.Sqrt
            )
            normc = apool.tile([P, ntiles], F32, tag="normc")
            nc.vector.tensor_scalar_max(out=normc[:], in0=norm[:], scalar1=EPS)
            inv = apool.tile([P, ntiles], F32, tag="inv")
            nc.vector.reciprocal(inv[:], normc[:])
            cos = apool.tile([P, ntiles], F32, tag="cos")
            nc.vector.tensor_mul(cos[:], dot_acc[:], inv[:])
            red = apool.tile([P, 1], F32, tag="red")
            red_i = nc.vector.tensor_reduce(
                red[:], cos[:], axis=mybir.AxisListType.X, op=mybir.AluOpType.add
            )
            # Pre-drain observers: the kernel-tail drain runs on the SP
            # proc, which otherwise observed nothing and would need one
            # wait per outstanding sem (over the 1-wait budget). NOP waits
            # execute on SP (unlike DMA-instruction waits, which run on the
            # DGE ring procs), so give SP single-wait nops observing every
            # DMA lane's final x tick, the last ACT op, and the last DVE
            # op. The out-DMA goes on the scalar queue (Act observed every
            # lane via the Squares, so it carries only its DVE data dep),
            # leaving the drain exactly one wait: the out-DMA's own lane.
            for d in dma_insts[-min(len(dma_insts), _X_BUFS):]:
                n_i = nc.sync.nop()
                tile.add_dep_helper(
                    n_i.ins, d.ins, sync=True, reason="drain lane observer"
                )
            n_i = nc.sync.nop()
            tile.add_dep_helper(
                n_i.ins, sqrt_i.ins, sync=True, reason="drain ACT observer"
            )
            n_i = nc.sync.nop()
            tile.add_dep_helper(
                n_i.ins, red_i.ins, sync=True, reason="drain DVE observer"
            )
            nc.scalar.dma_start(out_ext[:], red[:])
    return nc


def _shard_inputs(logits: np.ndarray, labels: np.ndarray, ntiles: int = NTILES):
    rows = ntiles * P
    iota = np.broadcast_to(np.arange(C, dtype=np.float32), (P, C))
    in_maps = []
    for k in range(N_CORES):
        lo = k * rows
        shard = np.ascontiguousarray(logits[lo:lo + rows], dtype=np.float32)
        lab = labels[lo:lo + rows].astype(np.float32).reshape(ntiles, P).T
        aux = np.concatenate([iota, lab], axis=1).astype(np.float32)
        in_maps.append({"logits": shard, "aux": np.ascontiguousarray(aux)})
    return in_maps


def _run(logits: np.ndarray, labels: np.ndarray, trace: bool = False):
    nc = build_nc()
    in_maps = _shard_inputs(logits, labels)
    res = run_bass_kernel_spmd(
        nc, in_maps, list(range(N_CORES)), trace=trace
    )
    total = 0.0
    for r in res.results:
        total += float(r["partial"].astype(np.float64).sum())
    loss = np.float32(1.0 - total / N)
    return np.asarray(loss, dtype=np.float32), res


def kernel(**inputs) -> np.ndarray:
    logits = np.asarray(inputs["logits"], dtype=np.float32)
    labels = np.asarray(inputs["labels"])
    out, _ = _run(logits, labels, trace=False)
    return out


# revision 27
# speedup vs baseline: 458.4692x; 458.4692x over previous
"""Cosine loss kernel for Trainium2 (8 NeuronCores, SPMD data-parallel).

loss = mean(1 - logits[i, labels[i]] / max(||logits[i]||, eps))

Sharding: rows split evenly across 8 cores. Each core streams its
[16384, 1000] f32 shard through SBUF in [128, 1000] tiles and computes,
per row i:
  dot_i  = logits[i, labels[i]]   (one fused DVE scalar_tensor_tensor:
                                   (iota == label_i) * x, accumulated)
  ssq_i  = sum(logits[i, :]^2)    (one ACT Square activation w/ accum)
then a small tail computes sum(cos_i) per partition. Host sums the 8
[128, 1] partials and finishes 1 - total/N.
"""

import os
import sys

import numpy as np

try:
    import concourse.bass as bass  # noqa: F401
except ImportError:
    for _p in ("/opt/trn_rl_repo", "/root/.axon_site/_ro/trn_rl_repo"):
        if os.path.isdir(_p) and _p not in sys.path:
            sys.path.insert(0, _p)
    import concourse.bass as bass

import concourse.mybir as mybir
from concourse import tile
from concourse.bass_utils import run_bass_kernel_spmd

N, C = 131072, 1000
N_CORES = 8
P = 128
ROWS_PER_CORE = N // N_CORES          # 16384
NTILES = ROWS_PER_CORE // P           # 128
EPS = 1e-8
F32 = mybir.dt.float32

_X_BUFS = 8


def build_nc(ntiles: int = NTILES, npasses: int = 1) -> bass.Bass:
    nc = bass.Bass()
    rows = ntiles * P
    logits_in = nc.declare_dram_parameter("logits", [rows, C], F32, isOutput=False)
    # aux = [iota (C cols) | labels (ntiles cols)] fused into one tensor so
    # the first STT only ever waits on 2 DMA sems (aux + x) — the HW
    # instruction encoding has a small sync-wait budget.
    aux_in = nc.declare_dram_parameter("aux", [P, C + ntiles], F32, isOutput=False)
    out_ext = nc.declare_dram_parameter("partial", [P, 1], F32, isOutput=True)

    with tile.TileContext(nc) as tc:
        with (
            tc.tile_pool(name="const", bufs=1) as cpool,
            tc.tile_pool(name="x", bufs=_X_BUFS) as xpool,
            tc.tile_pool(name="scratch", bufs=2) as spool,
            tc.tile_pool(name="acc", bufs=1) as apool,
        ):
            aux_t = cpool.tile([P, C + ntiles], F32, tag="aux")
            nc.sync.dma_start(aux_t[:], aux_in[:])
            iota_t = aux_t[:, :C]
            # Pre-touch aux on DVE: the STT encoding has a tiny sync-wait
            # budget, so let this copy absorb the aux-DMA wait — every STT
            # then only waits on its x-tile DMA.
            labels_t = cpool.tile([P, ntiles], F32, tag="labels")
            nc.vector.tensor_copy(labels_t[:], aux_t[:, C:])

            dot_acc = apool.tile([P, ntiles], F32, tag="dot")
            ssq_acc = apool.tile([P, ntiles], F32, tag="ssq")

            # Each HW instruction can encode at most ONE distinct wait-sem
            # (same-sem requirements merge; distinct sems don't). Loop
            # structure enforcing that invariant:
            #  - x-DMAs issue from the scalar sequencer (HWDGE waits run at
            #    the issuing sequencer, so all ACT-side release/WAR
            #    requirements are same-proc and free);
            #  - the DMA refilling a slot is issued right after the Square
            #    that last read it (same 8-lane rotation position, so the
            #    ring-reuse lane wait is already observed by that Square's
            #    own wait) and carries only the DVE WAR sem;
            #  - per-tile one-column "drain absorber" copies carry the
            #    engine-sem waits (reading the persistent accumulators, not
            #    pooled scratch, to stay out of slot-release bookkeeping),
            #    so STT/Square carry only their x-DMA lane wait.
            nsteps = ntiles * npasses
            xs = []
            dma_insts = []
            for t in range(min(_X_BUFS, nsteps)):
                u = t % ntiles
                x = xpool.tile([P, C], F32, tag="x")
                d = nc.scalar.dma_start(x[:], logits_in[u * P:(u + 1) * P, :])
                xs.append(x)
                dma_insts.append(d)
            # Lane priming: one-off scalar copies observing every DMA lane
            # sem up front, so the first ring-reuse refill has its lane
            # requirement pre-covered whatever the lane numbering is.
            prime = cpool.tile([P, len(xs) + 1], F32, tag="prime")
            for k, xk in enumerate(xs):
                nc.scalar.copy(prime[:, k:k + 1], xk[:, :1])
            # aux's DMA lane is part of the same global 8-lane rotation; the
            # first ring-reuse refill inherits it, so observe it here too.
            nc.scalar.copy(prime[:, len(xs):len(xs) + 1], aux_t[:, :1])

            for t in range(nsteps):
                u = t % ntiles
                up = (t - 1) % ntiles
                x = xs[t]
                pa = spool.tile([P, 1], F32, tag="pa")
                nc.vector.tensor_copy(
                    pa[:], labels_t[:, :1] if t == 0 else dot_acc[:, up:up + 1]
                )
                stt_scr = spool.tile([P, C], F32, tag="stt")
                nc.vector.scalar_tensor_tensor(
                    out=stt_scr[:],
                    in0=iota_t,
                    scalar=labels_t[:, u:u + 1],
                    in1=x[:],
                    op0=mybir.AluOpType.is_equal,
                    op1=mybir.AluOpType.mult,
                    accum_out=dot_acc[:, u:u + 1],
                )

                if t > 0:
                    pc = spool.tile([P, 1], F32, tag="pc")
                    pc_i = nc.scalar.copy(pc[:], ssq_acc[:, up:up + 1])
                    # Refill the slot freed at t-1: pc_t's Act-sem wait
                    # (>= Square_{t-1}) covers the WAR, and Square_{t-1}
                    # already observed that slot's DMA lane sem, so the
                    # refill carries only the DVE WAR sem.
                    tn = t - 1 + _X_BUFS
                    if tn < nsteps:
                        un = tn % ntiles
                        xn = xpool.tile([P, C], F32, tag="x")
                        dma_i = nc.scalar.dma_start(
                            xn[:], logits_in[un * P:(un + 1) * P, :]
                        )
                        tile.add_dep_helper(
                            dma_i.ins,
                            pc_i.ins,
                            sync=False,
                            reason="x-slot refill ordered after pc absorber",
                        )
                        xs.append(xn)
                        dma_insts.append(dma_i)
                act_scr = spool.tile([P, C], F32, tag="act")
                sq = nc.scalar.activation(
                    out=act_scr[:],
                    in_=x[:],
                    func=mybir.ActivationFunctionType.Square,
                    accum_out=ssq_acc[:, u:u + 1],
                )

            # Tail: cos = dot / max(sqrt(ssq), EPS); out = per-partition sum(cos)
            norm = apool.tile([P, ntiles], F32, tag="norm")
            sqrt_i = nc.scalar.activation(
                out=norm[:], in_=ssq_acc[:], func=mybir.ActivationFunctionType.Sqrt
            )
            normc = apool.tile([P, ntiles], F32, tag="normc")
            nc.vector.tensor_scalar_max(out=normc[:], in0=norm[:], scalar1=EPS)
            inv = apool.tile([P, ntiles], F32, tag="inv")
            nc.vector.reciprocal(inv[:], normc[:])
            cos = apool.tile([P, ntiles], F32, tag="cos")
            nc.vector.tensor_mul(cos[:], dot_acc[:], inv[:])
            red = apool.tile([P, 1], F32, tag="red")
            red_i = nc.vector.tensor_reduce(
                red[:], cos[:], axis=mybir.AxisListType.X, op=mybir.AluOpType.add
            )
            # Pre-drain observers: the kernel-tail drain runs on the SP
            # proc, which otherwise observed nothing and would need one
            # wait per outstanding sem (over the 1-wait budget). NOP waits
            # execute on SP (unlike DMA-instruction waits, which run on the
            # DGE ring procs), so give SP single-wait nops observing every
            # DMA lane's final x tick, the last ACT op, and the last DVE
            # op. The out-DMA goes on the scalar queue (Act observed every
            # lane via the Squares, so it carries only its DVE data dep),
            # leaving the drain exactly one wait: the out-DMA's own lane.
            for d in dma_insts[-min(len(dma_insts), _X_BUFS):]:
                n_i = nc.sync.nop()
                tile.add_dep_helper(
                    n_i.ins, d.ins, sync=True, reason="drain lane observer"
                )
            n_i = nc.sync.nop()
            tile.add_dep_helper(
                n_i.ins, sqrt_i.ins, sync=True, reason="drain ACT observer"
            )
            n_i = nc.sync.nop()
            tile.add_dep_helper(
                n_i.ins, red_i.ins, sync=True, reason="drain DVE observer"
            )
            nc.scalar.dma_start(out_ext[:], red[:])
    return nc


def _shard_inputs(logits: np.ndarray, labels: np.ndarray, ntiles: int = NTILES):
    rows = ntiles * P
    iota = np.broadcast_to(np.arange(C, dtype=np.float32), (P, C))
    in_maps = []
    for k in range(N_CORES):
        lo = k * rows
        shard = np.ascontiguousarray(logits[lo:lo + rows], dtype=np.float32)
        lab = labels[lo:lo + rows].astype(np.float32).reshape(ntiles, P).T
        aux = np.concatenate([iota, lab], axis=1).astype(np.float32)
        in_maps.append({"logits": shard, "aux": np.ascontiguousarray(aux)})
    return in_maps


def _run(logits: np.ndarray, labels: np.ndarray, trace: bool = False):
    nc = build_nc()
    in_maps = _shard_inputs(logits, labels)
    res = run_bass_kernel_spmd(
        nc, in_maps, list(range(N_CORES)), trace=trace
    )
    total = 0.0
    for r in res.results:
        total += float(r["partial"].astype(np.float64).sum())
    loss = np.float32(1.0 - total / N)
    return np.asarray(loss, dtype=np.float32), res


def kernel(**inputs) -> np.ndarray:
    logits = np.asarray(inputs["logits"], dtype=np.float32)
    labels = np.asarray(inputs["labels"])
    out, _ = _run(logits, labels, trace=False)
    return out
